# revision 30
# baseline (speedup 1.0000x reference)
"""Trainium2 Bass kernel for a dense transformer block (LN -> 16-head causal
attention -> proj+residual -> LN -> FFN+residual), B=8 data-parallel over 8
NeuronCores (one batch element per core).

Matmuls run in fp16 (10 explicit mantissa bits -- near float32r precision
at half the DMA/SBUF cost, full PE speed at any free dim).  The residual
stream, layer norms and softmax bookkeeping stay fp32.  LayerNorm gamma/beta are folded
into the adjacent projection weights on the host (exact algebra), and the
1/sqrt(E) attention scale is folded into wq.

Activation dataflow is transposed ([feature, token]) for the matmul chain;
scores are computed transposed ([key, query]) so softmax denominators come
out of the PV matmul via an appended ones-column in V, removing any need to
transpose the attention probabilities.

SBUF is managed with three rotating big pools (pool size is
sum-over-tags of bufs*slot, reserved from pool alloc to release, stack
per side):
  big2 (2 slots, 32.5KB ea): x_sb -> h_sb -> V_pad -> attnT -> x_re -> h2_sb
  big1 (3 slots, 32KB ea):   hT, QT, KT           (dies after attention)
  bigA (2 slots, right side): x2, h2T             (proj -> end)
"""

import os
import sys

sys.path.insert(0, "/opt/trn_rl_repo")

# a cpu-pinned jax would hide the NeuronCores from the PJRT execution path
if os.environ.get("JAX_PLATFORMS") == "cpu":
    os.environ.pop("JAX_PLATFORMS")

import numpy as np

import concourse.bacc as bacc
import concourse.mybir as mybir
import concourse.tile as tile
from concourse.bass_utils import run_bass_kernel_spmd
from concourse.masks import make_identity

F32 = mybir.dt.float32
F16 = mybir.dt.float16
AF = mybir.ActivationFunctionType
OP = mybir.AluOpType
AX = mybir.AxisListType

P = 128
E = 1024
H = 16
D = 64
T = 1024
B = 8
F = 4 * E
EC = E // P     # 8 chunks of features
FC = F // P     # 32 chunks of ffn hidden
TBn = T // P    # 8 token blocks of 128
LN_EPS = 1e-5
NEG = -1.0e9
SKIP_NORM = False
STOP_AFTER_ATTN = False
STOP_BEFORE_FFN = False
SIMPLE_ATTN = False


def _emit_ln(nc, pool, src, dst, name, eps_sb):
    """src: [128, TBn, E] fp32 -> dst: [128, TBn, E] fp32 layernormed per row."""
    sums = pool.tile([P, TBn], F32, tag=f"{name}_sums", name=f"{name}_sums")
    sumsq = pool.tile([P, TBn], F32, tag=f"{name}_sumsq", name=f"{name}_sumsq")
    for tb in range(TBn):
        nc.vector.reduce_sum(sums[:, tb : tb + 1], src[:, tb], axis=AX.X)
        dump = pool.tile([P, E], F32, tag=f"{name}_dump", name=f"{name}_dump")
        nc.scalar.activation(
            dump[:], src[:, tb], AF.Square, accum_out=sumsq[:, tb : tb + 1]
        )
    mu = pool.tile([P, TBn], F32, tag=f"{name}_mu", name=f"{name}_mu")
    var = pool.tile([P, TBn], F32, tag=f"{name}_var", name=f"{name}_var")
    rstd = pool.tile([P, TBn], F32, tag=f"{name}_rstd", name=f"{name}_rstd")
    nmr = pool.tile([P, TBn], F32, tag=f"{name}_nmr", name=f"{name}_nmr")
    nc.vector.tensor_scalar_mul(mu[:], sums[:], 1.0 / E)
    nc.vector.tensor_scalar_mul(var[:], sumsq[:], 1.0 / E)
    nc.vector.tensor_tensor(nmr[:], mu[:], mu[:], op=OP.mult)
    nc.vector.tensor_tensor(var[:], var[:], nmr[:], op=OP.subtract)
    # rstd = 1/sqrt(var + eps)
    nc.scalar.activation(rstd[:], var[:], AF.Sqrt, bias=eps_sb[:])
    nc.vector.reciprocal(rstd[:], rstd[:])
    nc.vector.tensor_tensor(nmr[:], mu[:], rstd[:], op=OP.mult)
    nc.vector.tensor_scalar_mul(nmr[:], nmr[:], -1.0)
    for tb in range(TBn):
        nc.scalar.activation(
            dst[:, tb],
            src[:, tb],
            AF.Identity,
            bias=nmr[:, tb : tb + 1],
            scale=rstd[:, tb : tb + 1],
        )


def _emit_transpose(nc, ps_tp, src, dst, ident, name):
    """src: [128, TBn, E] fp32 (token-major) -> dst: [128, EC, T] f32r
    (feature-major) via PE transposes of 128x128 blocks."""
    for j in range(EC):
        for tb in range(TBn):
            tp = ps_tp.tile([P, P], src.dtype, tag="tp", name=f"{name}_tp_{j}_{tb}")
            nc.tensor.transpose(tp[:], src[:, tb, j * P : (j + 1) * P], ident[:])
            if (j + tb) % 2 == 0:
                nc.vector.tensor_copy(dst[:, j, tb * P : (tb + 1) * P], tp[:])
            else:
                nc.scalar.copy(dst[:, j, tb * P : (tb + 1) * P], tp[:])


def build_nc(reps=1):
    nc = bacc.Bacc(None, target_bir_lowering=False)

    x_d = nc.dram_tensor("x", [T, E], F32, kind="ExternalInput")
    # weights pre-tiled on host to [out_chunk, p(=in%128), in_chunk, col]
    wqt_d = nc.dram_tensor("wqt", [EC, P, EC, P], F16, kind="ExternalInput")
    wkt_d = nc.dram_tensor("wkt", [EC, P, EC, P], F16, kind="ExternalInput")
    wvt_d = nc.dram_tensor("wvt", [EC, P, EC, P], F16, kind="ExternalInput")
    wpt_d = nc.dram_tensor("wpt", [EC, P, EC, P], F16, kind="ExternalInput")
    w1t_d = nc.dram_tensor("w1t", [FC, P, EC, P], F16, kind="ExternalInput")
    w2t_d = nc.dram_tensor("w2t", [EC, P, FC, P], F16, kind="ExternalInput")
    bq_d = nc.dram_tensor("bq", [P, EC], F32, kind="ExternalInput")
    bk_d = nc.dram_tensor("bk", [P, EC], F32, kind="ExternalInput")
    bp_d = nc.dram_tensor("bp", [P, EC], F32, kind="ExternalInput")
    b1_d = nc.dram_tensor("b1", [P, FC], F32, kind="ExternalInput")
    b2_d = nc.dram_tensor("b2", [P, EC], F32, kind="ExternalInput")
    out_d = nc.dram_tensor("out", [T, E], F32, kind="ExternalOutput")

    with tile.TileContext(nc) as tc:
      for _rep in range(reps):
        ps_big = tc.alloc_tile_pool(name="ps_big", bufs=3, space="PSUM")
        ps_attn = tc.alloc_tile_pool(name="ps_attn", bufs=2, space="PSUM")
        ps_tp = tc.alloc_tile_pool(name="ps_tp", bufs=2, space="PSUM")
        constp = tc.alloc_tile_pool(name="const", bufs=1)

        ident = constp.tile([P, P], F32, name="ident")
        make_identity(nc, ident[:])
        ident16 = constp.tile([P, P], F16, name="ident16")
        make_identity(nc, ident16[:])
        # mask[s, q] = 0 if s <= q else NEG  (within a diagonal 128 block)
        mask = constp.tile([P, P], F32, name="mask")
        nc.gpsimd.memset(mask[:], 0.0)
        nc.gpsimd.affine_select(
            out=mask[:],
            in_=mask[:],
            compare_op=OP.is_ge,
            fill=NEG,
            base=0,
            pattern=[[1, P]],
            channel_multiplier=-1,
        )
        eps_sb = constp.tile([P, 1], F32, name="eps_sb")
        nc.vector.memset(eps_sb[:], LN_EPS)
        bq_sb = constp.tile([P, EC], F32, name="bq_sb")
        bk_sb = constp.tile([P, EC], F32, name="bk_sb")
        bp_sb = constp.tile([P, EC], F32, name="bp_sb")
        b1_sb = constp.tile([P, FC], F32, name="b1_sb")
        b2_sb = constp.tile([P, EC], F32, name="b2_sb")
        nc.sync.dma_start(bq_sb[:], bq_d[:])
        nc.sync.dma_start(bk_sb[:], bk_d[:])
        nc.sync.dma_start(bp_sb[:], bp_d[:])
        nc.sync.dma_start(b1_sb[:], b1_d[:])
        nc.sync.dma_start(b2_sb[:], b2_d[:])

        big2 = tc.alloc_tile_pool(name="big2", bufs=3)
        big1 = tc.alloc_tile_pool(name="big1", bufs=1)

        # ---- Phase 1: LN1 + transpose ----
        ln1p = tc.alloc_tile_pool(name="ln1", bufs=1)
        x_sb = big2.tile([P, TBn, E], F32, tag="b2", name="x_sb")
        for tb in range(TBn):
            nc.sync.dma_start(x_sb[:, tb], x_d[tb * P : (tb + 1) * P, :])
        h_sb = big2.tile([P, TBn, E], F16, tag="b2", name="h_sb")
        hT = big1.tile([P, EC, T], F16, tag="b1", name="hT")
        _emit_ln(nc, ln1p, x_sb, h_sb, "ln1", eps_sb)
        _emit_transpose(nc, ps_tp, h_sb, hT, ident16, "h")
        ln1p.release()

        # release LN transpose psum before the merged phase (PSUM budget)
        ps_tp.release()

        # ---- Phase 2+3: QKV + attention, interleaved per head-quad ----
        # For each quad of 4 heads (2 feature chunks): compute QT/KT/V, then
        # run attention while draining next quad's QKV matmuls between
        # attention steps so the in-order PE never stalls on ACT's exps.
        ps_sc = tc.alloc_tile_pool(name="ps_sc", bufs=3, space="PSUM")
        wqkp = tc.alloc_tile_pool(name="wqk", bufs=3)
        qkp = tc.alloc_tile_pool(name="qkp", bufs=8)
        expp = tc.alloc_tile_pool(name="expp", bufs=6)
        rp = tc.alloc_tile_pool(name="rp", bufs=2)

        V_pad = big2.tile([P, TBn, H * (D + 1)], F16, tag="b2", name="V_pad")
        attnT = big2.tile([P, EC, T], F16, tag="b2", name="attnT")
        vpr = V_pad[:].rearrange("p tb (h dd) -> p tb h dd", dd=D + 1)
        ones16 = constp.tile([P, TBn, H], F32, name="ones16")
        nc.vector.memset(ones16[:], 1.0)
        nc.vector.tensor_copy(vpr[:, :, :, D : D + 1], ones16[:, :, :, None])

        def emit_qkv_quad(q):
            """DMA the quad's weights, allocate QT/KT tiles, and return
            (qt, kt, thunks); each thunk emits one psum accumulation group."""
            wq_t = wqkp.tile([P, 2, EC, P], F16, tag="wqk", name=f"wq_{_rep}_{q}")
            nc.sync.dma_start(
                wq_t[:],
                wqt_d[2 * q : 2 * q + 2].rearrange("jj p i c -> p jj i c"),
            )
            wk_t = wqkp.tile([P, 2, EC, P], F16, tag="wqk", name=f"wk_{_rep}_{q}")
            nc.sync.dma_start(
                wk_t[:],
                wkt_d[2 * q : 2 * q + 2].rearrange("jj p i c -> p jj i c"),
            )
            wv_t = wqkp.tile([P, 2, EC, P], F16, tag="wqk", name=f"wv_{_rep}_{q}")
            nc.sync.dma_start(
                wv_t[:],
                wvt_d[2 * q : 2 * q + 2].rearrange("jj p i c -> p jj i c"),
            )
            qt, kt, thunks = {}, {}, []
            for jj in range(2):
                j = 2 * q + jj
                qt[jj] = qkp.tile([P, T], F16, tag="qk", name=f"QT_{_rep}_{j}")
                kt[jj] = qkp.tile([P, T], F16, tag="qk", name=f"KT_{_rep}_{j}")
                for wsel, (w_t, dstt, bias_sb) in enumerate(
                    [(wq_t, qt[jj], bq_sb), (wk_t, kt[jj], bk_sb)]
                ):
                    for tq in range(2):
                        def _qk(w_t=w_t, dstt=dstt, bias_sb=bias_sb, jj=jj,
                                j=j, tq=tq, wsel=wsel):
                            psm = ps_big.tile(
                                [P, 512], F32, tag="mm",
                                name=f"qk_ps_{_rep}_{wsel}_{j}_{tq}",
                            )
                            for i in range(EC):
                                nc.tensor.matmul(
                                    psm[:],
                                    w_t[:, jj, i],
                                    hT[:, i, tq * 512 : (tq + 1) * 512],
                                    start=(i == 0),
                                    stop=(i == EC - 1),
                                )
                            nc.vector.tensor_scalar_add(
                                dstt[:, tq * 512 : (tq + 1) * 512],
                                psm[:],
                                bias_sb[:, j : j + 1],
                            )
                        thunks.append(_qk)
            for m in range(TBn):
                def _v(m=m, wv_t=wv_t, q=q):
                    psm = ps_big.tile(
                        [P, 512], F32, tag="mm", name=f"v_ps_{_rep}_{q}_{m}"
                    )
                    for i in range(EC):
                        nc.tensor.matmul(
                            psm[:, :256],
                            hT[:, i, m * P : (m + 1) * P],
                            wv_t[:, :, i, :],
                            start=(i == 0),
                            stop=(i == EC - 1),
                        )
                    dst4 = vpr[:, m, q * 4 : (q + 1) * 4, 0:D]
                    src4 = psm[:, :256].rearrange("p (h d) -> p h d", d=D)
                    nc.vector.tensor_copy(dst4, src4)
                thunks.append(_v)
            return qt, kt, thunks

        qt_cur, kt_cur, thunks0 = emit_qkv_quad(0)
        for t in thunks0:
            t()
        for q in range(4):
            if q < 3:
                qt_next, kt_next, bg = emit_qkv_quad(q + 1)
            else:
                qt_next = kt_next = None
                bg = []
            for t in bg:
                t()
            bg = []
            bgi = 0
            for jj in range(2):
                j = 2 * q + jj
                QTj, KTj = qt_cur[jj], kt_cur[jj]
                for qc in range(2):
                    nblk = 4 * qc + 4
                    aps = [
                        ps_attn.tile(
                            [D + 1, 512], F32, tag="attn",
                            name=f"at_ps_{_rep}_{j}_{qc}_{hh}",
                        )
                        for hh in range(2)
                    ]
                    exps = {}
                    offs = {}
                    for sb in range(nblk):
                        off = max(0, (sb - 4 * qc) * P)
                        w = 512 - off
                        offs[sb] = off
                        for hh in range(2):
                            sc = ps_sc.tile(
                                [P, 512], F32, tag="sc",
                                name=f"sc_{_rep}_{j}_{qc}_{sb}_{hh}",
                            )
                            nc.tensor.matmul(
                                sc[:, :w],
                                KTj[hh * D : (hh + 1) * D, sb * P : (sb + 1) * P],
                                QTj[hh * D : (hh + 1) * D,
                                    qc * 512 + off : (qc + 1) * 512],
                                start=True,
                                stop=True,
                                tile_position=None,
                            )
                            if sb >= 4 * qc:
                                nc.vector.tensor_tensor(
                                    sc[:, :P], sc[:, :P], mask[:], op=OP.add
                                )
                            ex = expp.tile(
                                [P, 512], F16, tag="exp",
                                name=f"ex_{_rep}_{j}_{qc}_{sb}_{hh}",
                            )
                            nc.scalar.activation(ex[:, :w], sc[:, :w], AF.Exp)
                            exps[(sb, hh)] = ex
                        # software-pipeline: PV of sb-1 issued after scores of sb
                        if sb > 0:
                            for hh in range(2):
                                po = offs[sb - 1]
                                nc.tensor.matmul(
                                    aps[hh][:, po:512],
                                    V_pad[:, sb - 1,
                                          (2 * j + hh) * (D + 1)
                                          : (2 * j + hh + 1) * (D + 1)],
                                    exps[(sb - 1, hh)][:, : 512 - po],
                                    start=(sb - 1 == 0),
                                    stop=False,
                                )
                        # drain one background QKV group for the next quad
                        if bgi < len(bg):
                            bg[bgi]()
                            bgi += 1
                    for hh in range(2):
                        po = offs[nblk - 1]
                        nc.tensor.matmul(
                            aps[hh][:, po:512],
                            V_pad[:, nblk - 1,
                                  (2 * j + hh) * (D + 1)
                                  : (2 * j + hh + 1) * (D + 1)],
                            exps[(nblk - 1, hh)][:, : 512 - po],
                            start=(nblk == 1),
                            stop=True,
                        )
                    for hh in range(2):
                        if SKIP_NORM:
                            nc.vector.tensor_copy(
                                attnT[hh * D : (hh + 1) * D, j,
                                      qc * 512 : (qc + 1) * 512],
                                aps[hh][0:D, :],
                            )
                            continue
                        rinv = rp.tile(
                            [1, 512], F32, tag="rinv", name=f"ri_{_rep}_{j}_{qc}_{hh}"
                        )
                        nc.vector.reciprocal(rinv[:], aps[hh][D : D + 1, :])
                        rb = rp.tile(
                            [D, 512], F32, tag="rb", name=f"rb_{_rep}_{j}_{qc}_{hh}"
                        )
                        nc.gpsimd.partition_broadcast(rb[:], rinv[:])
                        nc.vector.tensor_tensor(
                            attnT[hh * D : (hh + 1) * D, j,
                                  qc * 512 : (qc + 1) * 512],
                            aps[hh][0:D, :],
                            rb[:],
                            op=OP.mult,
                        )
            while bgi < len(bg):
                bg[bgi]()
                bgi += 1
            qt_cur, kt_cur = qt_next, kt_next
        rp.release()
        expp.release()
        qkp.release()
        wqkp.release()
        ps_sc.release()
        big1.release()
        ps_tp2 = tc.alloc_tile_pool(name="ps_tp2", bufs=2, space="PSUM")

        if STOP_AFTER_ATTN:
            for c in range(EC):
                nc.sync.dma_start(
                    out_d[c * P : (c + 1) * P, 0:512],
                    attnT[:, c, :].bitcast(F32),
                )
            big2.release()
            constp.release()
            ps_tp2.release()
            ps_attn.release()
            ps_big.release()
            continue

        # ---- Phase 4: proj + residual ----
        pA_x2 = tc.alloc_tile_pool(name="pA_x2", bufs=1, side="right")
        wpp = tc.alloc_tile_pool(name="wpp", bufs=2)
        satp = tc.alloc_tile_pool(name="satp", bufs=2)
        x2 = pA_x2.tile([P, TBn, E], F32, tag="bA_x2", name="x2")
        for c in range(EC):
            if c % 4 == 0:
                wcol = wpp.tile([P, 4, EC, P], F16, tag="wp", name=f"wp_{c // 4}")
                nc.sync.dma_start(
                    wcol[:],
                    wpt_d[c : c + 4].rearrange("jj p i cc -> p jj i cc"),
                )
            saT = satp.tile([P, T], F32, tag="saT", name=f"saT_{c}")
            for tq in range(2):
                psm = ps_big.tile([P, 512], F32, tag="mm", name=f"pj_ps_{c}_{tq}")
                for i in range(EC):
                    nc.tensor.matmul(
                        psm[:],
                        wcol[:, c % 4, i],
                        attnT[:, i, tq * 512 : (tq + 1) * 512],
                        start=(i == 0),
                        stop=(i == EC - 1),
                    )
                nc.scalar.activation(
                    saT[:, tq * 512 : (tq + 1) * 512],
                    psm[:],
                    AF.Identity,
                    bias=bp_sb[:, c : c + 1],
                )
            for tb in range(TBn):
                tp = ps_tp2.tile([P, P], F32, tag="tp", name=f"pj_tp_{c}_{tb}")
                nc.tensor.transpose(tp[:], saT[:, tb * P : (tb + 1) * P], ident[:])
                nc.vector.tensor_tensor(
                    x2[:, tb, c * P : (c + 1) * P],
                    x_sb[:, tb, c * P : (c + 1) * P],
                    tp[:],
                    op=OP.add,
                )
        satp.release()
        wpp.release()

        # ---- Phase 5: LN2 + transpose ----
        pA_h2 = tc.alloc_tile_pool(name="pA_h2", bufs=1, side="right")
        ln2p = tc.alloc_tile_pool(name="ln2", bufs=1)
        h2T = pA_h2.tile([P, EC, T], F16, tag="bA_h2T", name="h2T")
        h2_sb = big2.tile([P, TBn, E], F16, tag="b2", name="h2_sb")
        _emit_ln(nc, ln2p, x2, h2_sb, "ln2", eps_sb)
        _emit_transpose(nc, ps_tp2, h2_sb, h2T, ident16, "h2")
        ln2p.release()
        big2.release()

        if STOP_BEFORE_FFN:
            for c in range(EC):
                nc.sync.dma_start(
                    out_d[c * P : (c + 1) * P, 0:512],
                    h2T[:, c, :].bitcast(F32),
                )
            pA_h2.release()
            pA_x2.release()
            constp.release()
            ps_tp2.release()
            ps_attn.release()
            ps_big.release()
            continue

        # ---- Phase 6: FFN + residual + output ----
        # joint token-halves: h1T holds the full hidden state so w1 and w2
        # each stream from HBM exactly once
        h1p = tc.alloc_tile_pool(name="h1p", bufs=1)
        fw1 = tc.alloc_tile_pool(name="fw1", bufs=2)
        h1T = h1p.tile([P, FC, T], F16, tag="h1T", name="h1T")
        for k in range(FC):
            if k % 4 == 0:
                w1col = fw1.tile(
                    [P, 4, EC, P], F16, tag="w1col", name=f"w1c_{k // 4}"
                )
                nc.sync.dma_start(
                    w1col[:],
                    w1t_d[k : k + 4].rearrange("kk p i c -> p kk i c"),
                )
            for th in range(2):
                psm = ps_big.tile([P, 512], F32, tag="mm", name=f"h1_ps_{k}_{th}")
                for i in range(EC):
                    nc.tensor.matmul(
                        psm[:],
                        w1col[:, k % 4, i],
                        h2T[:, i, th * 512 : (th + 1) * 512],
                        start=(i == 0),
                        stop=(i == EC - 1),
                    )
                nc.scalar.activation(
                    h1T[:, k, th * 512 : (th + 1) * 512],
                    psm[:],
                    AF.Relu,
                    bias=b1_sb[:, k : k + 1],
                )
        fw1.release()
        pA_h2.release()
        fw2 = tc.alloc_tile_pool(name="fw2", bufs=2)
        fout = tc.alloc_tile_pool(name="fout", bufs=1)
        fftp = tc.alloc_tile_pool(name="fftp", bufs=2)
        ostage = fout.tile([P, TBn, E], F32, tag="ostage", name="ostage")
        for c in range(EC):
            w2col = fw2.tile([P, FC, P], F16, tag="w2col", name=f"w2c_{c}")
            nc.sync.dma_start(w2col[:], w2t_d[c])
            for th in range(2):
                psm = ps_big.tile([P, 512], F32, tag="mm", name=f"ff_ps_{c}_{th}")
                for k in range(FC):
                    nc.tensor.matmul(
                        psm[:],
                        w2col[:, k],
                        h1T[:, k, th * 512 : (th + 1) * 512],
                        start=(k == 0),
                        stop=(k == FC - 1),
                    )
                ffT = fftp.tile([P, 512], F32, tag="ffT", name=f"ffT_{c}_{th}")
                nc.scalar.activation(
                    ffT[:], psm[:], AF.Identity, bias=b2_sb[:, c : c + 1]
                )
                for tbl in range(4):
                    tb = th * 4 + tbl
                    tp = ps_tp2.tile([P, P], F32, tag="tp", name=f"f_tp_{c}_{th}_{tbl}")
                    nc.tensor.transpose(
                        tp[:], ffT[:, tbl * P : (tbl + 1) * P], ident[:]
                    )
                    nc.vector.tensor_tensor(
                        ostage[:, tb, c * P : (c + 1) * P],
                        x2[:, tb, c * P : (c + 1) * P],
                        tp[:],
                        op=OP.add,
                    )
            # flush finished output halves early to overlap the store
            if c == 3 or c == EC - 1:
                half = 0 if c == 3 else 1
                for tb in range(TBn):
                    nc.sync.dma_start(
                        out_d[tb * P : (tb + 1) * P, half * 512 : (half + 1) * 512],
                        ostage[:, tb, half * 512 : (half + 1) * 512],
                    )
        fftp.release()
        fout.release()
        fw2.release()
        h1p.release()
        pA_x2.release()
        constp.release()
        ps_tp2.release()
        ps_attn.release()
        ps_big.release()

    nc.compile()
    return nc


_NC = None


def _get_nc():
    global _NC
    if _NC is None:
        _NC = build_nc()
    return _NC


def prepare_in_maps(x, wq, wk, wv, w_proj, b_proj, w1, b1, w2, b2,
                    ln1_g, ln1_b, ln2_g, ln2_b):
    x = np.asarray(x, dtype=np.float32)
    wq2 = np.asarray(wq, dtype=np.float32).reshape(E, E)
    wk2 = np.asarray(wk, dtype=np.float32).reshape(E, E)
    wv2 = np.asarray(wv, dtype=np.float32).reshape(E, E)
    w_proj = np.asarray(w_proj, dtype=np.float32)
    b_proj = np.asarray(b_proj, dtype=np.float32)
    w1 = np.asarray(w1, dtype=np.float32)
    b1 = np.asarray(b1, dtype=np.float32)
    w2 = np.asarray(w2, dtype=np.float32)
    b2 = np.asarray(b2, dtype=np.float32)
    g1 = np.asarray(ln1_g, dtype=np.float32)
    be1 = np.asarray(ln1_b, dtype=np.float32)
    g2 = np.asarray(ln2_g, dtype=np.float32)
    be2 = np.asarray(ln2_b, dtype=np.float32)

    def _tile_w(arr):
        # [K_in, N_out] -> [N_out//P, P(=k_in%P), K_in//P, P] so each DMA reads
        # contiguous per-partition lines
        K, N = arr.shape
        return np.ascontiguousarray(
            arr.reshape(K // P, P, N // P, P).transpose(2, 1, 0, 3)
        )

    scale = np.float32(E) ** -0.5
    # fold LN1 gamma into qkv weights, LN1 beta into qkv biases; fold the
    # attention scale into wq.  V's bias is constant across tokens after
    # softmax (rows sum to 1), so it folds into the proj bias.
    wqt = np.ascontiguousarray((wq2 * g1[None, :] * scale).T)
    wkt = np.ascontiguousarray((wk2 * g1[None, :]).T)
    wvt = np.ascontiguousarray((wv2 * g1[None, :]).T)
    bq = (wq2 @ be1) * scale
    bk = wk2 @ be1
    bv = wv2 @ be1
    wpt = np.ascontiguousarray(w_proj.T)
    bp = b_proj + w_proj @ bv
    w1t = np.ascontiguousarray((w1 * g2[None, :]).T)
    b1e = b1 + w1 @ be2
    w2t = np.ascontiguousarray(w2.T)

    common = {
        "wqt": _tile_w(wqt.astype(np.float16)),
        "wkt": _tile_w(wkt.astype(np.float16)),
        "wvt": _tile_w(wvt.astype(np.float16)),
        "wpt": _tile_w(wpt.astype(np.float16)),
        "w1t": _tile_w(w1t.astype(np.float16)),
        "w2t": _tile_w(w2t.astype(np.float16)),
        "bq": np.ascontiguousarray(bq.reshape(EC, P).T),
        "bk": np.ascontiguousarray(bk.reshape(EC, P).T),
        "bp": np.ascontiguousarray(bp.reshape(EC, P).T),
        "b1": np.ascontiguousarray(b1e.reshape(FC, P).T),
        "b2": np.ascontiguousarray(b2.reshape(EC, P).T),
    }
    return [dict(common, x=np.ascontiguousarray(x[b])) for b in range(B)]


def kernel(**inputs):
    in_maps = prepare_in_maps(**inputs)
    nc = _get_nc()
    res = run_bass_kernel_spmd(nc, in_maps, core_ids=list(range(B)))
    out = np.stack([res.results[b]["out"] for b in range(B)], axis=0)
    return out.astype(np.float32)


# revision 31
# speedup vs baseline: 1.0779x; 1.0779x over previous
"""Trainium2 Bass kernel for a dense transformer block (LN -> 16-head causal
attention -> proj+residual -> LN -> FFN+residual), B=8 data-parallel over 8
NeuronCores (one batch element per core).

Matmuls run in fp16 (10 explicit mantissa bits -- near float32r precision
at half the DMA/SBUF cost, full PE speed at any free dim).  The residual
stream, layer norms and softmax bookkeeping stay fp32.  LayerNorm gamma/beta are folded
into the adjacent projection weights on the host (exact algebra), and the
1/sqrt(E) attention scale is folded into wq.

Activation dataflow is transposed ([feature, token]) for the matmul chain;
scores are computed transposed ([key, query]) so softmax denominators come
out of the PV matmul via an appended ones-column in V, removing any need to
transpose the attention probabilities.

SBUF is managed with three rotating big pools (pool size is
sum-over-tags of bufs*slot, reserved from pool alloc to release, stack
per side):
  big2 (2 slots, 32.5KB ea): x_sb -> h_sb -> V_pad -> attnT -> x_re -> h2_sb
  big1 (3 slots, 32KB ea):   hT, QT, KT           (dies after attention)
  bigA (2 slots, right side): x2, h2T             (proj -> end)
"""

import os
import sys

sys.path.insert(0, "/opt/trn_rl_repo")

# a cpu-pinned jax would hide the NeuronCores from the PJRT execution path
if os.environ.get("JAX_PLATFORMS") == "cpu":
    os.environ.pop("JAX_PLATFORMS")

import numpy as np

import concourse.bacc as bacc
import concourse.mybir as mybir
import concourse.tile as tile
from concourse.bass_utils import run_bass_kernel_spmd
from concourse.masks import make_identity

F32 = mybir.dt.float32
F16 = mybir.dt.float16
AF = mybir.ActivationFunctionType
OP = mybir.AluOpType
AX = mybir.AxisListType

P = 128
E = 1024
H = 16
D = 64
T = 1024
B = 8
F = 4 * E
EC = E // P     # 8 chunks of features
FC = F // P     # 32 chunks of ffn hidden
TBn = T // P    # 8 token blocks of 128
LN_EPS = 1e-5
NEG = -1.0e9
SKIP_NORM = False
STOP_AFTER_ATTN = False
STOP_BEFORE_FFN = False
SIMPLE_ATTN = False


def _emit_ln(nc, pool, src, dst, name, eps_sb):
    """src: [128, TBn, E] fp32 -> dst: [128, TBn, E] fp32 layernormed per row."""
    sums = pool.tile([P, TBn], F32, tag=f"{name}_sums", name=f"{name}_sums")
    sumsq = pool.tile([P, TBn], F32, tag=f"{name}_sumsq", name=f"{name}_sumsq")
    for tb in range(TBn):
        nc.vector.reduce_sum(sums[:, tb : tb + 1], src[:, tb], axis=AX.X)
        dump = pool.tile([P, E], F32, tag=f"{name}_dump", name=f"{name}_dump")
        nc.scalar.activation(
            dump[:], src[:, tb], AF.Square, accum_out=sumsq[:, tb : tb + 1]
        )
    mu = pool.tile([P, TBn], F32, tag=f"{name}_mu", name=f"{name}_mu")
    var = pool.tile([P, TBn], F32, tag=f"{name}_var", name=f"{name}_var")
    rstd = pool.tile([P, TBn], F32, tag=f"{name}_rstd", name=f"{name}_rstd")
    nmr = pool.tile([P, TBn], F32, tag=f"{name}_nmr", name=f"{name}_nmr")
    nc.vector.tensor_scalar_mul(mu[:], sums[:], 1.0 / E)
    nc.vector.tensor_scalar_mul(var[:], sumsq[:], 1.0 / E)
    nc.vector.tensor_tensor(nmr[:], mu[:], mu[:], op=OP.mult)
    nc.vector.tensor_tensor(var[:], var[:], nmr[:], op=OP.subtract)
    # rstd = 1/sqrt(var + eps)
    nc.scalar.activation(rstd[:], var[:], AF.Sqrt, bias=eps_sb[:])
    nc.vector.reciprocal(rstd[:], rstd[:])
    nc.vector.tensor_tensor(nmr[:], mu[:], rstd[:], op=OP.mult)
    nc.vector.tensor_scalar_mul(nmr[:], nmr[:], -1.0)
    for tb in range(TBn):
        nc.scalar.activation(
            dst[:, tb],
            src[:, tb],
            AF.Identity,
            bias=nmr[:, tb : tb + 1],
            scale=rstd[:, tb : tb + 1],
        )


def _emit_transpose(nc, ps_tp, src, dst, ident, name):
    """src: [128, TBn, E] fp32 (token-major) -> dst: [128, EC, T] f32r
    (feature-major) via PE transposes of 128x128 blocks."""
    for j in range(EC):
        for tb in range(TBn):
            tp = ps_tp.tile([P, P], src.dtype, tag="tp", name=f"{name}_tp_{j}_{tb}")
            nc.tensor.transpose(tp[:], src[:, tb, j * P : (j + 1) * P], ident[:])
            if (j + tb) % 2 == 0:
                nc.vector.tensor_copy(dst[:, j, tb * P : (tb + 1) * P], tp[:])
            else:
                nc.scalar.copy(dst[:, j, tb * P : (tb + 1) * P], tp[:])


def build_nc(reps=1):
    nc = bacc.Bacc(None, target_bir_lowering=False)

    x_d = nc.dram_tensor("x", [T, E], F32, kind="ExternalInput")
    # weights pre-tiled on host to [out_chunk, p(=in%128), in_chunk, col]
    wqt_d = nc.dram_tensor("wqt", [EC, P, EC, P], F16, kind="ExternalInput")
    wkt_d = nc.dram_tensor("wkt", [EC, P, EC, P], F16, kind="ExternalInput")
    wvt_d = nc.dram_tensor("wvt", [EC, P, EC, P], F16, kind="ExternalInput")
    wpt_d = nc.dram_tensor("wpt", [EC, P, EC, P], F16, kind="ExternalInput")
    w1t_d = nc.dram_tensor("w1t", [FC, P, EC, P], F16, kind="ExternalInput")
    w2t_d = nc.dram_tensor("w2t", [EC, P, FC, P], F16, kind="ExternalInput")
    bq_d = nc.dram_tensor("bq", [P, EC], F32, kind="ExternalInput")
    bk_d = nc.dram_tensor("bk", [P, EC], F32, kind="ExternalInput")
    bp_d = nc.dram_tensor("bp", [P, EC], F32, kind="ExternalInput")
    b1_d = nc.dram_tensor("b1", [P, FC], F32, kind="ExternalInput")
    b2_d = nc.dram_tensor("b2", [P, EC], F32, kind="ExternalInput")
    out_d = nc.dram_tensor("out", [T, E], F32, kind="ExternalOutput")

    with tile.TileContext(nc) as tc:
      for _rep in range(reps):
        ps_big = tc.alloc_tile_pool(name="ps_big", bufs=2, space="PSUM")
        ps_attn = tc.alloc_tile_pool(name="ps_attn", bufs=2, space="PSUM")
        ps_tp = tc.alloc_tile_pool(name="ps_tp", bufs=2, space="PSUM")
        constp = tc.alloc_tile_pool(name="const", bufs=1)

        ident = constp.tile([P, P], F32, name="ident")
        make_identity(nc, ident[:])
        ident16 = constp.tile([P, P], F16, name="ident16")
        make_identity(nc, ident16[:])
        # mask[s, q] = 0 if s <= q else NEG  (within a diagonal 128 block)
        mask = constp.tile([P, P], F32, name="mask")
        nc.gpsimd.memset(mask[:], 0.0)
        nc.gpsimd.affine_select(
            out=mask[:],
            in_=mask[:],
            compare_op=OP.is_ge,
            fill=NEG,
            base=0,
            pattern=[[1, P]],
            channel_multiplier=-1,
        )
        eps_sb = constp.tile([P, 1], F32, name="eps_sb")
        nc.vector.memset(eps_sb[:], LN_EPS)
        bq_sb = constp.tile([P, EC], F32, name="bq_sb")
        bk_sb = constp.tile([P, EC], F32, name="bk_sb")
        bp_sb = constp.tile([P, EC], F32, name="bp_sb")
        b1_sb = constp.tile([P, FC], F32, name="b1_sb")
        b2_sb = constp.tile([P, EC], F32, name="b2_sb")
        nc.sync.dma_start(bq_sb[:], bq_d[:])
        nc.sync.dma_start(bk_sb[:], bk_d[:])
        nc.sync.dma_start(bp_sb[:], bp_d[:])
        nc.sync.dma_start(b1_sb[:], b1_d[:])
        nc.sync.dma_start(b2_sb[:], b2_d[:])

        big2 = tc.alloc_tile_pool(name="big2", bufs=3)
        big1 = tc.alloc_tile_pool(name="big1", bufs=1)

        # ---- Phase 1: LN1 + transpose ----
        ln1p = tc.alloc_tile_pool(name="ln1", bufs=1)
        x_sb = big2.tile([P, TBn, E], F32, tag="b2", name="x_sb")
        for tb in range(TBn):
            nc.sync.dma_start(x_sb[:, tb], x_d[tb * P : (tb + 1) * P, :])
        h_sb = big2.tile([P, TBn, E], F16, tag="b2", name="h_sb")
        hT = big1.tile([P, EC, T], F16, tag="b1", name="hT")
        _emit_ln(nc, ln1p, x_sb, h_sb, "ln1", eps_sb)
        _emit_transpose(nc, ps_tp, h_sb, hT, ident16, "h")
        ln1p.release()

        # release LN transpose psum before the merged phase (PSUM budget)
        ps_tp.release()

        # ---- Phase 2+3: QKV + attention, interleaved per head-quad ----
        # For each quad of 4 heads (2 feature chunks): compute QT/KT/V, then
        # run attention while draining next quad's QKV matmuls between
        # attention steps so the in-order PE never stalls on ACT's exps.
        ps_sc = tc.alloc_tile_pool(name="ps_sc", bufs=2, space="PSUM")
        wqkp = tc.alloc_tile_pool(name="wqk", bufs=3)
        qkp = tc.alloc_tile_pool(name="qkp", bufs=8)
        expp = tc.alloc_tile_pool(name="expp", bufs=6)
        rp = tc.alloc_tile_pool(name="rp", bufs=2)

        V_pad = big2.tile([P, TBn, H * (D + 1)], F16, tag="b2", name="V_pad")
        attnT = big2.tile([P, EC, T], F16, tag="b2", name="attnT")
        vpr = V_pad[:].rearrange("p tb (h dd) -> p tb h dd", dd=D + 1)
        ones16 = constp.tile([P, TBn, H], F32, name="ones16")
        nc.vector.memset(ones16[:], 1.0)
        nc.vector.tensor_copy(vpr[:, :, :, D : D + 1], ones16[:, :, :, None])

        def emit_qkv_quad(q):
            """DMA the quad's weights, allocate QT/KT tiles, and return
            (qt, kt, thunks); each thunk emits one psum accumulation group."""
            wq_t = wqkp.tile([P, 2, EC, P], F16, tag="wqk", name=f"wq_{_rep}_{q}")
            nc.sync.dma_start(
                wq_t[:],
                wqt_d[2 * q : 2 * q + 2].rearrange("jj p i c -> p jj i c"),
            )
            wk_t = wqkp.tile([P, 2, EC, P], F16, tag="wqk", name=f"wk_{_rep}_{q}")
            nc.sync.dma_start(
                wk_t[:],
                wkt_d[2 * q : 2 * q + 2].rearrange("jj p i c -> p jj i c"),
            )
            wv_t = wqkp.tile([P, 2, EC, P], F16, tag="wqk", name=f"wv_{_rep}_{q}")
            nc.sync.dma_start(
                wv_t[:],
                wvt_d[2 * q : 2 * q + 2].rearrange("jj p i c -> p jj i c"),
            )
            qt, kt, thunks = {}, {}, []
            for jj in range(2):
                j = 2 * q + jj
                qt[jj] = qkp.tile([P, T], F16, tag="qk", name=f"QT_{_rep}_{j}")
                kt[jj] = qkp.tile([P, T], F16, tag="qk", name=f"KT_{_rep}_{j}")
                for wsel, (w_t, dstt, bias_sb) in enumerate(
                    [(wq_t, qt[jj], bq_sb), (wk_t, kt[jj], bk_sb)]
                ):
                    for tq in range(2):
                        def _qk(w_t=w_t, dstt=dstt, bias_sb=bias_sb, jj=jj,
                                j=j, tq=tq, wsel=wsel):
                            psm = ps_big.tile(
                                [P, 512], F32, tag="mm",
                                name=f"qk_ps_{_rep}_{wsel}_{j}_{tq}",
                            )
                            for i in range(EC):
                                nc.tensor.matmul(
                                    psm[:],
                                    w_t[:, jj, i],
                                    hT[:, i, tq * 512 : (tq + 1) * 512],
                                    start=(i == 0),
                                    stop=(i == EC - 1),
                                )
                            nc.vector.tensor_scalar_add(
                                dstt[:, tq * 512 : (tq + 1) * 512],
                                psm[:],
                                bias_sb[:, j : j + 1],
                            )
                        thunks.append(_qk)
            for m in range(TBn):
                def _v(m=m, wv_t=wv_t, q=q):
                    psm = ps_big.tile(
                        [P, 512], F32, tag="mm", name=f"v_ps_{_rep}_{q}_{m}"
                    )
                    for i in range(EC):
                        nc.tensor.matmul(
                            psm[:, :256],
                            hT[:, i, m * P : (m + 1) * P],
                            wv_t[:, :, i, :],
                            start=(i == 0),
                            stop=(i == EC - 1),
                        )
                    dst4 = vpr[:, m, q * 4 : (q + 1) * 4, 0:D]
                    src4 = psm[:, :256].rearrange("p (h d) -> p h d", d=D)
                    nc.vector.tensor_copy(dst4, src4)
                thunks.append(_v)
            return qt, kt, thunks

        qt_cur, kt_cur, thunks0 = emit_qkv_quad(0)
        for t in thunks0:
            t()
        for q in range(4):
            if q < 3:
                qt_next, kt_next, bg = emit_qkv_quad(q + 1)
            else:
                qt_next = kt_next = None
                bg = []
            for t in bg:
                t()
            bg = []
            bgi = 0
            for jj in range(2):
                j = 2 * q + jj
                QTj, KTj = qt_cur[jj], kt_cur[jj]
                for qc in range(2):
                    nblk = 4 * qc + 4
                    aps = [
                        ps_attn.tile(
                            [D + 1, 512], F32, tag="attn",
                            name=f"at_ps_{_rep}_{j}_{qc}_{hh}",
                        )
                        for hh in range(2)
                    ]
                    exps = {}
                    offs = {}
                    for sb in range(nblk):
                        off = max(0, (sb - 4 * qc) * P)
                        w = 512 - off
                        offs[sb] = off
                        # both heads share one 2-bank psum tile: h0 cols
                        # [off,512), h1 cols [512+off,1024) -> one fused mask
                        # add and one fused exp per step instead of two
                        sc = ps_sc.tile(
                            [P, 1024], F32, tag="sc",
                            name=f"sc_{_rep}_{j}_{qc}_{sb}",
                        )
                        for hh in range(2):
                            nc.tensor.matmul(
                                sc[:, hh * 512 + off : (hh + 1) * 512],
                                KTj[hh * D : (hh + 1) * D, sb * P : (sb + 1) * P],
                                QTj[hh * D : (hh + 1) * D,
                                    qc * 512 + off : (qc + 1) * 512],
                                start=True,
                                stop=True,
                                tile_position=None,
                            )
                        scv = sc[:].rearrange("p (g c) -> p g c", g=2)
                        if sb >= 4 * qc:
                            nc.vector.tensor_tensor(
                                scv[:, :, off : off + P],
                                scv[:, :, off : off + P],
                                mask[:, None, :].to_broadcast([P, 2, P]),
                                op=OP.add,
                            )
                        ex = expp.tile(
                            [P, 1024], F16, tag="exp",
                            name=f"ex_{_rep}_{j}_{qc}_{sb}",
                        )
                        exv = ex[:].rearrange("p (g c) -> p g c", g=2)
                        nc.scalar.activation(
                            exv[:, :, off:512], scv[:, :, off:512], AF.Exp
                        )
                        exps[sb] = ex
                        # software-pipeline: PV of sb-1 issued after scores of sb
                        if sb > 0:
                            for hh in range(2):
                                po = offs[sb - 1]
                                nc.tensor.matmul(
                                    aps[hh][:, po:512],
                                    V_pad[:, sb - 1,
                                          (2 * j + hh) * (D + 1)
                                          : (2 * j + hh + 1) * (D + 1)],
                                    exps[sb - 1][:, hh * 512 + po
                                                 : (hh + 1) * 512],
                                    start=(sb - 1 == 0),
                                    stop=False,
                                )
                        # drain one background QKV group for the next quad
                        if bgi < len(bg):
                            bg[bgi]()
                            bgi += 1
                    for hh in range(2):
                        po = offs[nblk - 1]
                        nc.tensor.matmul(
                            aps[hh][:, po:512],
                            V_pad[:, nblk - 1,
                                  (2 * j + hh) * (D + 1)
                                  : (2 * j + hh + 1) * (D + 1)],
                            exps[nblk - 1][:, hh * 512 + po : (hh + 1) * 512],
                            start=(nblk == 1),
                            stop=True,
                        )
                    for hh in range(2):
                        if SKIP_NORM:
                            nc.vector.tensor_copy(
                                attnT[hh * D : (hh + 1) * D, j,
                                      qc * 512 : (qc + 1) * 512],
                                aps[hh][0:D, :],
                            )
                            continue
                        rinv = rp.tile(
                            [1, 512], F32, tag="rinv", name=f"ri_{_rep}_{j}_{qc}_{hh}"
                        )
                        nc.vector.reciprocal(rinv[:], aps[hh][D : D + 1, :])
                        rb = rp.tile(
                            [D, 512], F32, tag="rb", name=f"rb_{_rep}_{j}_{qc}_{hh}"
                        )
                        nc.gpsimd.partition_broadcast(rb[:], rinv[:])
                        nc.vector.tensor_tensor(
                            attnT[hh * D : (hh + 1) * D, j,
                                  qc * 512 : (qc + 1) * 512],
                            aps[hh][0:D, :],
                            rb[:],
                            op=OP.mult,
                        )
            while bgi < len(bg):
                bg[bgi]()
                bgi += 1
            qt_cur, kt_cur = qt_next, kt_next
        rp.release()
        expp.release()
        qkp.release()
        wqkp.release()
        ps_sc.release()
        big1.release()
        ps_tp2 = tc.alloc_tile_pool(name="ps_tp2", bufs=2, space="PSUM")

        if STOP_AFTER_ATTN:
            for c in range(EC):
                nc.sync.dma_start(
                    out_d[c * P : (c + 1) * P, 0:512],
                    attnT[:, c, :].bitcast(F32),
                )
            big2.release()
            constp.release()
            ps_tp2.release()
            ps_attn.release()
            ps_big.release()
            continue

        # ---- Phase 4: proj + residual ----
        pA_x2 = tc.alloc_tile_pool(name="pA_x2", bufs=1, side="right")
        wpp = tc.alloc_tile_pool(name="wpp", bufs=2)
        satp = tc.alloc_tile_pool(name="satp", bufs=2)
        x2 = pA_x2.tile([P, TBn, E], F32, tag="bA_x2", name="x2")
        for c in range(EC):
            if c % 4 == 0:
                wcol = wpp.tile([P, 4, EC, P], F16, tag="wp", name=f"wp_{c // 4}")
                nc.sync.dma_start(
                    wcol[:],
                    wpt_d[c : c + 4].rearrange("jj p i cc -> p jj i cc"),
                )
            saT = satp.tile([P, T], F32, tag="saT", name=f"saT_{c}")
            for tq in range(2):
                psm = ps_big.tile([P, 512], F32, tag="mm", name=f"pj_ps_{c}_{tq}")
                for i in range(EC):
                    nc.tensor.matmul(
                        psm[:],
                        wcol[:, c % 4, i],
                        attnT[:, i, tq * 512 : (tq + 1) * 512],
                        start=(i == 0),
                        stop=(i == EC - 1),
                    )
                nc.scalar.activation(
                    saT[:, tq * 512 : (tq + 1) * 512],
                    psm[:],
                    AF.Identity,
                    bias=bp_sb[:, c : c + 1],
                )
            for tb in range(TBn):
                tp = ps_tp2.tile([P, P], F32, tag="tp", name=f"pj_tp_{c}_{tb}")
                nc.tensor.transpose(tp[:], saT[:, tb * P : (tb + 1) * P], ident[:])
                nc.vector.tensor_tensor(
                    x2[:, tb, c * P : (c + 1) * P],
                    x_sb[:, tb, c * P : (c + 1) * P],
                    tp[:],
                    op=OP.add,
                )
        satp.release()
        wpp.release()

        # ---- Phase 5: LN2 + transpose ----
        pA_h2 = tc.alloc_tile_pool(name="pA_h2", bufs=1, side="right")
        ln2p = tc.alloc_tile_pool(name="ln2", bufs=1)
        h2T = pA_h2.tile([P, EC, T], F16, tag="bA_h2T", name="h2T")
        h2_sb = big2.tile([P, TBn, E], F16, tag="b2", name="h2_sb")
        _emit_ln(nc, ln2p, x2, h2_sb, "ln2", eps_sb)
        _emit_transpose(nc, ps_tp2, h2_sb, h2T, ident16, "h2")
        ln2p.release()
        big2.release()

        if STOP_BEFORE_FFN:
            for c in range(EC):
                nc.sync.dma_start(
                    out_d[c * P : (c + 1) * P, 0:512],
                    h2T[:, c, :].bitcast(F32),
                )
            pA_h2.release()
            pA_x2.release()
            constp.release()
            ps_tp2.release()
            ps_attn.release()
            ps_big.release()
            continue

        # ---- Phase 6: FFN + residual + output ----
        # joint token-halves: h1T holds the full hidden state so w1 and w2
        # each stream from HBM exactly once
        h1p = tc.alloc_tile_pool(name="h1p", bufs=1)
        fw1 = tc.alloc_tile_pool(name="fw1", bufs=2)
        h1T = h1p.tile([P, FC, T], F16, tag="h1T", name="h1T")
        for k in range(FC):
            if k % 4 == 0:
                w1col = fw1.tile(
                    [P, 4, EC, P], F16, tag="w1col", name=f"w1c_{k // 4}"
                )
                nc.sync.dma_start(
                    w1col[:],
                    w1t_d[k : k + 4].rearrange("kk p i c -> p kk i c"),
                )
            for th in range(2):
                psm = ps_big.tile([P, 512], F32, tag="mm", name=f"h1_ps_{k}_{th}")
                for i in range(EC):
                    nc.tensor.matmul(
                        psm[:],
                        w1col[:, k % 4, i],
                        h2T[:, i, th * 512 : (th + 1) * 512],
                        start=(i == 0),
                        stop=(i == EC - 1),
                    )
                nc.scalar.activation(
                    h1T[:, k, th * 512 : (th + 1) * 512],
                    psm[:],
                    AF.Relu,
                    bias=b1_sb[:, k : k + 1],
                )
        fw1.release()
        pA_h2.release()
        fw2 = tc.alloc_tile_pool(name="fw2", bufs=2)
        fout = tc.alloc_tile_pool(name="fout", bufs=1)
        fftp = tc.alloc_tile_pool(name="fftp", bufs=2)
        ostage = fout.tile([P, TBn, E], F32, tag="ostage", name="ostage")
        for c in range(EC):
            w2col = fw2.tile([P, FC, P], F16, tag="w2col", name=f"w2c_{c}")
            nc.sync.dma_start(w2col[:], w2t_d[c])
            for th in range(2):
                psm = ps_big.tile([P, 512], F32, tag="mm", name=f"ff_ps_{c}_{th}")
                for k in range(FC):
                    nc.tensor.matmul(
                        psm[:],
                        w2col[:, k],
                        h1T[:, k, th * 512 : (th + 1) * 512],
                        start=(k == 0),
                        stop=(k == FC - 1),
                    )
                ffT = fftp.tile([P, 512], F32, tag="ffT", name=f"ffT_{c}_{th}")
                nc.scalar.activation(
                    ffT[:], psm[:], AF.Identity, bias=b2_sb[:, c : c + 1]
                )
                for tbl in range(4):
                    tb = th * 4 + tbl
                    tp = ps_tp2.tile([P, P], F32, tag="tp", name=f"f_tp_{c}_{th}_{tbl}")
                    nc.tensor.transpose(
                        tp[:], ffT[:, tbl * P : (tbl + 1) * P], ident[:]
                    )
                    nc.vector.tensor_tensor(
                        ostage[:, tb, c * P : (c + 1) * P],
                        x2[:, tb, c * P : (c + 1) * P],
                        tp[:],
                        op=OP.add,
                    )
            # flush finished output halves early to overlap the store
            if c == 3 or c == EC - 1:
                half = 0 if c == 3 else 1
                for tb in range(TBn):
                    nc.sync.dma_start(
                        out_d[tb * P : (tb + 1) * P, half * 512 : (half + 1) * 512],
                        ostage[:, tb, half * 512 : (half + 1) * 512],
                    )
        fftp.release()
        fout.release()
        fw2.release()
        h1p.release()
        pA_x2.release()
        constp.release()
        ps_tp2.release()
        ps_attn.release()
        ps_big.release()

    nc.compile()
    return nc


_NC = None


def _get_nc():
    global _NC
    if _NC is None:
        _NC = build_nc()
    return _NC


def prepare_in_maps(x, wq, wk, wv, w_proj, b_proj, w1, b1, w2, b2,
                    ln1_g, ln1_b, ln2_g, ln2_b):
    x = np.asarray(x, dtype=np.float32)
    wq2 = np.asarray(wq, dtype=np.float32).reshape(E, E)
    wk2 = np.asarray(wk, dtype=np.float32).reshape(E, E)
    wv2 = np.asarray(wv, dtype=np.float32).reshape(E, E)
    w_proj = np.asarray(w_proj, dtype=np.float32)
    b_proj = np.asarray(b_proj, dtype=np.float32)
    w1 = np.asarray(w1, dtype=np.float32)
    b1 = np.asarray(b1, dtype=np.float32)
    w2 = np.asarray(w2, dtype=np.float32)
    b2 = np.asarray(b2, dtype=np.float32)
    g1 = np.asarray(ln1_g, dtype=np.float32)
    be1 = np.asarray(ln1_b, dtype=np.float32)
    g2 = np.asarray(ln2_g, dtype=np.float32)
    be2 = np.asarray(ln2_b, dtype=np.float32)

    def _tile_w(arr):
        # [K_in, N_out] -> [N_out//P, P(=k_in%P), K_in//P, P] so each DMA reads
        # contiguous per-partition lines
        K, N = arr.shape
        return np.ascontiguousarray(
            arr.reshape(K // P, P, N // P, P).transpose(2, 1, 0, 3)
        )

    scale = np.float32(E) ** -0.5
    # fold LN1 gamma into qkv weights, LN1 beta into qkv biases; fold the
    # attention scale into wq.  V's bias is constant across tokens after
    # softmax (rows sum to 1), so it folds into the proj bias.
    wqt = np.ascontiguousarray((wq2 * g1[None, :] * scale).T)
    wkt = np.ascontiguousarray((wk2 * g1[None, :]).T)
    wvt = np.ascontiguousarray((wv2 * g1[None, :]).T)
    bq = (wq2 @ be1) * scale
    bk = wk2 @ be1
    bv = wv2 @ be1
    wpt = np.ascontiguousarray(w_proj.T)
    bp = b_proj + w_proj @ bv
    w1t = np.ascontiguousarray((w1 * g2[None, :]).T)
    b1e = b1 + w1 @ be2
    w2t = np.ascontiguousarray(w2.T)

    common = {
        "wqt": _tile_w(wqt.astype(np.float16)),
        "wkt": _tile_w(wkt.astype(np.float16)),
        "wvt": _tile_w(wvt.astype(np.float16)),
        "wpt": _tile_w(wpt.astype(np.float16)),
        "w1t": _tile_w(w1t.astype(np.float16)),
        "w2t": _tile_w(w2t.astype(np.float16)),
        "bq": np.ascontiguousarray(bq.reshape(EC, P).T),
        "bk": np.ascontiguousarray(bk.reshape(EC, P).T),
        "bp": np.ascontiguousarray(bp.reshape(EC, P).T),
        "b1": np.ascontiguousarray(b1e.reshape(FC, P).T),
        "b2": np.ascontiguousarray(b2.reshape(EC, P).T),
    }
    return [dict(common, x=np.ascontiguousarray(x[b])) for b in range(B)]


def kernel(**inputs):
    in_maps = prepare_in_maps(**inputs)
    nc = _get_nc()
    res = run_bass_kernel_spmd(nc, in_maps, core_ids=list(range(B)))
    out = np.stack([res.results[b]["out"] for b in range(B)], axis=0)
    return out.astype(np.float32)
